# revision 55
# baseline (speedup 1.0000x reference)
"""Trainium2 Bass kernel for nn_AttentionITBlock — v3 (zero on-chip transposes).

Contract: kernel(**inputs) takes FULL unsharded inputs, returns FULL output
(B, C, H, W) float32. Data-parallel over batch B=8, one batch per core.

Key ideas vs v2 (273us on HW):
  - Host ships the basis in BOTH contraction layouts as fp16: `bs` [hw, s]
    for the forward transform and `bsT` [s, hw] for the inverse. This
    removes all 648 on-chip PE transposes + 144 PSUM->SBUF copies that
    dominated v2's phase 3, at the cost of +21MB/core DMA (~51us at the
    measured 412GB/s/core, fully overlappable with attention + phase 3).
  - x also ships pre-transposed (xT) for the forward; the [C, HW] copy is
    streamed per-block for the shortcut. No PE transposes anywhere.
  - The inverse emits [hw, c] tiles (basis stationary, yh moving), so the
    LayerNorm reduction is along the FREE axis: one DVE square + one grouped
    tensor_reduce per 512-pixel block replaces v2's per-block ones-matmul /
    transpose / broadcast-matmul machinery (216 tiny PE ops gone). rstd is
    the same fp32 bit-hack + 2 Newton steps; normalization is a per-partition
    tensor_scalar; gamma/beta apply via pre-broadcast [128, 512] tiles.
  - Mixer W2 is folded into the attention output projection on the host
    (yh tiles come straight from ot), and biases (b2, scb, v-proj) fold into
    k=1 ones-row matmuls.
  - Attention: V^T produced directly by matmul (lhsT=Tcat tile), all 4 heads
    interleaved per token tile to fill the ACT exp latency, and the softmax
    1/sum uses reciprocal_approx_fast (1 DVE op) instead of the 1.9us
    iterative reciprocal.
  - Output is stored as [128, NBLK, 4, C] fp16 (partition-major pixels) and
    transposed back to [C, H, W] on the host.

PSUM banks: phase1 ph0-2 (3x [C,384]); attention po0-3 (4x [65,512]),
po64all [65,256], pl/pl64/pv rings; phase3 pm ping-pong (2x [128,4,128]w512),
psc ping-pong, pg staging for gamma/beta tiles.
"""

import sys

sys.path.insert(0, "/opt/trn_rl_repo")

import numpy as np

import concourse.bass as bass
import concourse.mybir as mybir
import concourse.tile as tile
from concourse import bacc
from concourse.bass_utils import run_bass_kernel_spmd

F32 = mybir.dt.float32
F16 = mybir.dt.float16
AF = mybir.ActivationFunctionType
I32 = mybir.dt.int32
ALU = mybir.AluOpType

B, C, H, W = 8, 128, 96, 96
HW = H * W                      # 9216
M1 = M2 = 24
S = M1 * M2                     # 576 tokens
SCAT = 2 * S                    # 1152 = re|im concatenated token axis
NH, DH = 4, 32
SCALE = 1.0 / np.sqrt(np.float32(DH))

NCH = HW // 128                 # 72 hw chunks of 128
NBLK = HW // 512                # 18 hw blocks of 512
NT = SCAT // 128                # 9 token tiles over the concatenated axis
TT = [(i * 128, min(128, S - i * 128)) for i in range((S + 127) // 128)]  # 5
MAGIC = 0x5F3759DF


DEBUG_DUMPS = False


def build_module(gelu=AF.Gelu_apprx_tanh):
    nc = bacc.Bacc("TRN2", target_bir_lowering=False, debug=False)

    d_x = nc.dram_tensor("x16", [C, HW], F16, kind="ExternalInput").ap()
    d_xT = nc.dram_tensor("xT16", [128, NCH, C], F16, kind="ExternalInput").ap()
    d_bs = nc.dram_tensor("bs16", [128, NCH, SCAT], F16, kind="ExternalInput").ap()
    d_bsT = nc.dram_tensor("bsT16", [128, NBLK, NT, 512], F16,
                           kind="ExternalInput").ap()
    d_wqkv = nc.dram_tensor("wqkv", [C, 12, C], F16, kind="ExternalInput").ap()
    d_bqkv = nc.dram_tensor("bqkv", [C, 4], F32, kind="ExternalInput").ap()
    d_bvrow = nc.dram_tensor("bvrow", [2, C], F16, kind="ExternalInput").ap()
    d_wo2 = nc.dram_tensor("wo2", [64, 8, C], F16, kind="ExternalInput").ap()
    d_byh = nc.dram_tensor("byh", [2, C], F16, kind="ExternalInput").ap()
    d_b2 = nc.dram_tensor("b2", [C], F32, kind="ExternalInput").ap()
    d_gam = nc.dram_tensor("gam", [C], F32, kind="ExternalInput").ap()
    d_bet = nc.dram_tensor("bet", [C], F32, kind="ExternalInput").ap()
    d_scw = nc.dram_tensor("scwT", [C, C], F16, kind="ExternalInput").ap()
    d_scb = nc.dram_tensor("scb", [C], F32, kind="ExternalInput").ap()
    d_out = nc.dram_tensor("out16", [C, HW], F16, kind="ExternalOutput").ap()
    dbg = None
    if DEBUG_DUMPS:
        dbg = {
            "Tcat": nc.dram_tensor("dbg_Tcat", [C, SCAT], F16,
                                   kind="ExternalOutput").ap(),
            "QP0": nc.dram_tensor("dbg_QP0", [C, S], F16,
                                  kind="ExternalOutput").ap(),
            "KP0": nc.dram_tensor("dbg_KP0", [C, S], F16,
                                  kind="ExternalOutput").ap(),
            "VH00": nc.dram_tensor("dbg_VH00", [128, 130], F16,
                                   kind="ExternalOutput").ap(),
            "OT": nc.dram_tensor("dbg_OT", [65, 4, S], F16,
                                 kind="ExternalOutput").ap(),
            "YH": nc.dram_tensor("dbg_YH", [128, NT, C], F16,
                                 kind="ExternalOutput").ap(),
        }

    with tile.TileContext(nc) as tc:
        _body(nc, tc, d_x, d_xT, d_bs, d_bsT, d_wqkv, d_bqkv, d_bvrow, d_wo2,
              d_byh, d_b2, d_gam, d_bet, d_scw, d_scb, d_out, gelu, dbg)
    nc.finalize()
    return nc


def _body(nc, tc, d_x, d_xT, d_bs, d_bsT, d_wqkv, d_bqkv, d_bvrow, d_wo2,
          d_byh, d_b2, d_gam, d_bet, d_scw, d_scb, d_out, gelu, dbg=None):
    from contextlib import ExitStack
    ctx = ExitStack()
    singles = ctx.enter_context(tc.tile_pool(name="singles", bufs=1))
    work = ctx.enter_context(tc.tile_pool(name="work", bufs=2))
    small = ctx.enter_context(tc.tile_pool(name="small", bufs=1))
    ps = ctx.enter_context(tc.tile_pool(name="ps", bufs=1, space="PSUM"))

    # ---------------- constants ----------------
    from concourse.masks import make_identity
    ident_f = work.tile([128, 128], F32, tag="identf", name="ident_f")
    make_identity(nc, ident_f)
    ident16 = singles.tile([128, 128], F16, tag="ident16")
    nc.vector.tensor_copy(ident16, ident_f)
    ones16 = singles.tile([128, 128], F16, tag="ones16")
    nc.vector.memset(ones16, 1.0)

    # ---------------- small weights (scalar queue; bulk loads go on sync) ----
    wqkv = singles.tile([C, 12, C], F16)
    nc.scalar.dma_start(wqkv, d_wqkv)
    wo2 = singles.tile([64, 8, C], F16)
    nc.scalar.dma_start(wo2, d_wo2)
    scw = singles.tile([C, C], F16, tag="scw")
    nc.scalar.dma_start(scw, d_scw)
    bqkv = singles.tile([C, 4], F32, tag="bqkv")
    nc.scalar.dma_start(bqkv, d_bqkv)
    # bias rows must each sit at partition 0 (matmul rhs with k=1)
    bvrow = []
    byh = []
    for p in range(2):
        t = singles.tile([1, C], F16, tag=f"bv{p}", name=f"bv{p}")
        nc.scalar.dma_start(t, d_bvrow[p:p + 1, :])
        bvrow.append(t)
        t = singles.tile([1, C], F16, tag=f"byh{p}", name=f"byh{p}")
        nc.scalar.dma_start(t, d_byh[p:p + 1, :])
        byh.append(t)
    def load_col(ap1d, nm):
        t = singles.tile([C, 1], F32, tag=nm, name=nm)
        nc.scalar.dma_start(t, ap1d[:, None])
        return t

    b2c = load_col(d_b2, "b2c")
    gam = load_col(d_gam, "gam")
    bet = load_col(d_bet, "bet")
    scb = load_col(d_scb, "scb")

    # ---------------- phase 1: forward transform ----------------
    # Stream xT + bs pieces of 4 chunks; 3 psum accumulators cover SCAT=1152.
    # The piece buffers are sized exactly like the phase-3 block buffers
    # (9216B / 1KB per partition), so both phases share one deep ring via
    # common tags — full temporal SBUF reuse.
    NP = 18
    per = NCH // NP  # 4
    RB = 12

    def load_piece(q):
        sl = slice(q * per, (q + 1) * per)
        xb = work.tile([128, per, C], F16, tag="xblk", bufs=RB, name="xpiece")
        nc.sync.dma_start(xb, d_xT[:, sl, :])
        bsb = work.tile([128, per, SCAT], F16, tag="bsT", bufs=RB,
                        name="bspiece")
        nc.sync.dma_start(bsb, d_bs[:, sl, :])
        return xb, bsb

    ph = [ps.tile([C, 384], F32, tag="ABC"[i], name=f"ph{i}", bufs=1)
          for i in range(3)]
    cur = load_piece(0)
    for q in range(NP):
        nxt = load_piece(q + 1) if q + 1 < NP else None
        xb, bsb = cur
        for j in range(per):
            k = per * q + j
            st = dict(start=(k == 0), stop=(k == NCH - 1))
            for i in range(3):
                nc.tensor.matmul(ph[i], xb[:, j, :],
                                 bsb[:, j, 384 * i:384 * (i + 1)], **st)
        cur = nxt

    Tcat = singles.tile([C, SCAT], F16, tag="Tcat")
    for i in range(3):
        nc.any.tensor_copy(Tcat[:, 384 * i:384 * (i + 1)], ph[i])
    if dbg:
        nc.gpsimd.dma_start(dbg["Tcat"], Tcat)

    # ---- prefetch phase-3 streams now (DMA queue drains during attention) --
    bsT_t = {}
    xblk_t = {}

    def prefetch_blk(k):
        t = work.tile([128, NT, 512], F16, tag="bsT", bufs=RB)
        nc.sync.dma_start(t, d_bsT[:, k, :, :])
        bsT_t[k] = t
        xb = work.tile([C, 512], F16, tag="xblk", bufs=RB)
        nc.sync.dma_start(xb, d_x[:, 512 * k:512 * (k + 1)])
        xblk_t[k] = xb

    for k in range(RB):
        prefetch_blk(k)

    # ---------------- phase 2: attention ----------------
    # PSUM: po0-3 in banks A-D, pl ring in E (2 bufs), bank G holds the po64
    # accumulators (cols 0:256; nothing else may write bank G while that
    # accumulation group is open — interleaved same-bank writes corrupt it)
    # plus the pre-loop pp64 staging (cols 256:512, temporally disjoint).
    # Bank H holds all loop-time 64-col transients: pl64 ping/pong (0:128),
    # pv ping/pong (128:384), rb64 ping/pong (384:512).
    gbank = ps.tile([128, 512], F32, tag="G", name="gbank", bufs=1)
    hbank = ps.tile([128, 512], F32, tag="H", name="hbank", bufs=1)

    # Projections: QP/KP per pair as [C, S] (packed [h0re|h0im|h1re|h1im]
    # rows); V^T directly via matmul into vh tiles [tsz, 130]
    # (= [h0(64) | one | h1(64) | one]).
    QP, KP = {}, {}
    for p in range(2):
        for i in range(2):  # 0=q, 1=k
            dst = singles.tile([C, S], F16, tag=f"qk{i}{p}", name=f"qk{i}{p}")
            iw = (i * 2 + p) * 2
            pp = ps.tile([C, 512], F32, tag="E", name="pp", bufs=2)
            nc.tensor.matmul(pp, wqkv[:, iw, :], Tcat[:, 0:512],
                             start=True, stop=False)
            nc.tensor.matmul(pp, wqkv[:, iw + 1, :], Tcat[:, S:S + 512],
                             start=False, stop=True)
            with nc.allow_low_precision(reason="fp16 qk"):
                nc.vector.tensor_scalar(dst[:, 0:512], pp,
                                        bqkv[:, i * 2 + p:i * 2 + p + 1],
                                        None, ALU.add)
            g0 = 256 + 64 * (i * 2 + p)  # 256..448, disjoint per proj
            pp64 = gbank[:, g0:g0 + 64]
            nc.tensor.matmul(pp64, wqkv[:, iw, :], Tcat[:, 512:576],
                             start=True, stop=False)
            nc.tensor.matmul(pp64, wqkv[:, iw + 1, :], Tcat[:, S + 512:S + 576],
                             start=False, stop=True)
            with nc.allow_low_precision(reason="fp16 qk"):
                nc.vector.tensor_scalar(dst[:, 512:576], pp64,
                                        bqkv[:, i * 2 + p:i * 2 + p + 1],
                                        None, ALU.add)
            (QP if i == 0 else KP)[p] = dst
    if dbg:
        nc.gpsimd.dma_start(dbg["QP0"], QP[0])
        nc.gpsimd.dma_start(dbg["KP0"], KP[0])

    vh = {}
    for p in range(2):
        for ti, (t0, tsz) in enumerate(TT):
            iw = (4 + p) * 2
            g0 = 128 + 128 * ((p * len(TT) + ti) % 2)
            pv = hbank[:, g0:g0 + 128]
            nc.tensor.matmul(pv[:tsz], Tcat[:, t0:t0 + tsz], wqkv[:, iw, :],
                             start=True, stop=False)
            nc.tensor.matmul(pv[:tsz], Tcat[:, S + t0:S + t0 + tsz],
                             wqkv[:, iw + 1, :], start=False, stop=False)
            nc.tensor.matmul(pv[:tsz], ones16[0:1, 0:tsz],
                             bvrow[p], start=False, stop=True)
            t = singles.tile([128, 130], F16, tag=f"vh{p}{ti}",
                             name=f"vh{p}{ti}")
            nc.vector.memset(t, 1.0)
            with nc.allow_low_precision(reason="fp16 v"):
                nc.vector.tensor_copy(t[:tsz, 0:64], pv[:tsz, 0:64])
                nc.scalar.copy(t[:tsz, 65:129], pv[:tsz, 64:128])
            vh[(p, ti)] = t
    if dbg:
        nc.gpsimd.dma_start(dbg["VH00"], vh[(0, 0)])

    # 4-head interleaved attention
    po = {h: ps.tile([65, 512], F32, tag="ABCD"[h], name=f"po{h}", bufs=1)
          for h in range(4)}
    for ti, (t0, tsz) in enumerate(TT):
        ats = {}
        for h in range(4):
            p, sub = h // 2, h % 2
            hsl = slice(64 * sub, 64 * sub + 64)
            tp = (64, 0) if sub else None
            at = work.tile([128, S], F16, tag="attnT", bufs=4)
            pl = ps.tile([128, 512], F32, tag="E", name="pl", bufs=2)
            nc.tensor.matmul(pl[:tsz], KP[p][hsl, t0:t0 + tsz],
                             QP[p][hsl, 0:512],
                             start=True, stop=True, tile_position=tp)
            nc.scalar.activation(at[:tsz, 0:512], pl[:tsz], AF.Exp,
                                 scale=float(SCALE))
            g0 = 64 * (h % 2)
            pl64 = hbank[:, g0:g0 + 64]
            nc.tensor.matmul(pl64[:tsz], KP[p][hsl, t0:t0 + tsz],
                             QP[p][hsl, 512:576],
                             start=True, stop=True, tile_position=tp)
            nc.scalar.activation(at[:tsz, 512:576], pl64[:tsz], AF.Exp,
                                 scale=float(SCALE))
            ats[h] = at
        for h in range(4):
            p, sub = h // 2, h % 2
            at = ats[h]
            lh = vh[(p, ti)][:tsz, 65 * sub:65 * sub + 65]
            nc.tensor.matmul(po[h], lh, at[:tsz, 0:512],
                             start=(ti == 0), stop=(ti == len(TT) - 1))
            # all 4 tails share bank G: exactly ONE start (whole bank goes
            # lazy-zero; each head's first write then stores) and ONE stop.
            nc.tensor.matmul(gbank[0:65, 64 * h:64 * h + 64], lh,
                             at[:tsz, 512:576],
                             start=(ti == 0 and h == 0),
                             stop=(ti == len(TT) - 1 and h == 3))

    # softmax 1/sum + normalize -> ot[h] [65, S] f16 (rows 0:64 normalized)
    ots = []
    for h in range(4):
        ot = singles.tile([65, S], F16, tag=f"ot{h}", name=f"ot{h}")
        with nc.allow_low_precision(reason="fp16 sums"):
            nc.scalar.copy(ot[64:65, 0:512], po[h][64:65])
            nc.scalar.copy(ot[64:65, 512:576],
                           gbank[64:65, 64 * h:64 * h + 64])
        rb = ps.tile([64, 512], F32, tag="E", name="rb", bufs=2)
        nc.tensor.matmul(rb, ones16[64:65, 0:64], ot[64:65, 0:512],
                         start=True, stop=True, tile_position=(64, 0))
        g0 = 384 + 64 * (h % 2)
        rb64 = hbank[0:64, g0:g0 + 64]
        nc.tensor.matmul(rb64, ones16[64:65, 0:64], ot[64:65, 512:576],
                         start=True, stop=True, tile_position=(64, 0))
        rcp = work.tile([64, 512], F32, tag="rcp", name="rcp", bufs=2)
        nc.vector.reciprocal_approx_fast(rcp, rb)
        rcp64 = small.tile([64, 64], F32, tag=f"rcp64_{h}", bufs=1)
        nc.vector.reciprocal_approx_fast(rcp64, rb64)
        with nc.allow_low_precision(reason="fp16 softmax weights"):
            nc.vector.tensor_mul(ot[0:64, 0:512], po[h][0:64], rcp)
            nc.vector.tensor_mul(ot[0:64, 512:576],
                                 gbank[0:64, 64 * h:64 * h + 64], rcp64)
        ots.append(ot)
    if dbg:
        for h in range(4):
            nc.gpsimd.dma_start(dbg["OT"][:, h, :], ots[h])

    # yh fold: yh[s', o] = sum_h ot[h]^T @ wo2 (+ byh bias row), tiles over
    # the 1152 concat axis. Tile 4 straddles re|im at col 576: two 64-row
    # segments, the second writing psum partitions 64:128 (tile_position
    # auto-derives the M offset from out.base_partition()).
    yh = singles.tile([128, NT, C], F16, tag="yh")
    for t in range(NT):
        pyh = ps.tile([128, C], F32, tag="E", name="pyh", bufs=2)
        lo = 128 * t
        if lo < S < lo + 128:
            segs = [(0, lo, S - lo, 0), (S - lo, 0, lo + 128 - S, 1)]
        elif lo < S:
            segs = [(0, lo, 128, 0)]
        else:
            segs = [(0, lo - S, 128, 1)]
        for (m0, s0, mlen, part) in segs:
            out = pyh[m0:m0 + mlen, :]
            for h in range(NH):
                nc.tensor.matmul(out, ots[h][0:64, s0:s0 + mlen],
                                 wo2[:, part * 4 + h, :],
                                 start=(h == 0), stop=False)
            nc.tensor.matmul(out, ones16[0:1, 0:mlen], byh[part],
                             start=False, stop=True)
        with nc.allow_low_precision(reason="fp16 yh"):
            nc.vector.tensor_copy(yh[:, t, :], pyh)
    if dbg:
        nc.gpsimd.dma_start(dbg["YH"], yh)

    # ---------------- phase 3: inverse transform + LN tail ----------------
    # [c, hw] orientation: yh tiles are the (small) stationary weights, bsT
    # streams through the moving port at 2.4GHz. LN stats: one ones-column
    # matmul per block emits the variance directly as a ROW (at psum
    # partition 32q of bank H); the rstd bit-hack chain then runs on FOUR
    # blocks at once over a partition-strided [4, 512] view, and each
    # block's rstd row broadcasts to [C, 512] with a single k=1 matmul
    # (lhsT = ones row at the matching partition). gamma/beta/b2/scb ride
    # per-partition ACT bias/scale for free.
    GROUPS = [list(range(g, g + 3)) for g in range(0, NBLK, 3)]
    state = {}
    state2 = {}
    ydict = {}

    def blk_start(kb):
        if kb + RB < NBLK:
            prefetch_blk(kb + RB)
        bsT = bsT_t.pop(kb)
        xblk = xblk_t.pop(kb)
        pm = ps.tile([C, 512], F32, tag="AB"[kb % 2], name="pm", bufs=1)
        for t in range(NT):
            nc.tensor.matmul(pm, yh[:, t, :], bsT[:, t, :],
                             start=(t == 0), stop=(t == NT - 1))
        mh = work.tile([C, 512], F16, tag="mh", bufs=7)
        nc.scalar.activation(mh, pm, AF.Identity, bias=b2c, scale=1.0)
        sq = work.tile([C, 512], F16, tag="sq", bufs=5)
        with nc.allow_low_precision(reason="fp16 m^2 for variance"):
            nc.gpsimd.tensor_mul(sq, mh, mh)
        state[kb] = (mh, sq, xblk)

    def blk_mid(kb):
        mh, sq, xblk = state.pop(kb)
        psc = ps.tile([C, 512], F32, tag="C", name="psc", bufs=1)
        nc.tensor.matmul(psc, scw, xblk, start=True, stop=True)
        psc16 = work.tile([C, 512], F16, tag="psc16", bufs=7)
        with nc.allow_low_precision(reason="fp16 shortcut"):
            nc.vector.tensor_copy(psc16, psc)
        vcol = hbank[:, 4 * (kb % 3):4 * (kb % 3) + 4]
        for j in range(4):
            nc.tensor.matmul(vcol[:, j:j + 1], sq[:, 128 * j:128 * (j + 1)],
                             ones16[:, 0:1], start=(j == 0), stop=(j == 3))
        state2[kb] = (mh, psc16)

    def grp_chain(g):
        n = 4 * len(GROUPS[g])
        view = hbank[:, 0:n]
        v4 = work.tile([128, 12], F32, tag="cv4", bufs=1, name="cv4")[:, 0:n]
        w1 = work.tile([128, 12], F32, tag="cw1", bufs=1, name="cw1")[:, 0:n]
        w2 = work.tile([128, 12], F32, tag="cw2", bufs=1, name="cw2")[:, 0:n]
        w3 = work.tile([128, 12], F32, tag="cw3", bufs=1, name="cw3")[:, 0:n]
        nc.vector.tensor_scalar(v4, view, 1.0 / C, 1e-5, ALU.mult, ALU.add)
        nc.vector.tensor_scalar(w1.bitcast(I32), v4.bitcast(I32), 1, None,
                                ALU.logical_shift_right)
        nc.vector.tensor_scalar(w2.bitcast(I32), w1.bitcast(I32), -1, MAGIC,
                                ALU.mult, ALU.add)
        y = w2
        nc.vector.tensor_mul(w3, v4, y)
        nc.vector.tensor_mul(w1, w3, y)
        nc.vector.tensor_scalar(w3, w1, -0.5, 1.5, ALU.mult, ALU.add)
        yn = w1
        nc.vector.tensor_mul(yn, y, w3)
        y16g = work.tile([128, 12], F16, tag="y16g", bufs=2)
        with nc.allow_low_precision(reason="fp16 rstd"):
            nc.vector.tensor_copy(y16g[:, 0:n], yn)
        ydict[g] = y16g

    def blk_finish(kb):
        mh, psc16 = state2.pop(kb)
        q = 4 * (kb % 3)
        y16g = ydict[kb // 3]
        prr = ps.tile([1, 512], F16, tag="E", name="prr", bufs=2)
        for j in range(4):
            nc.tensor.transpose(prr[:, 128 * j:128 * (j + 1)],
                                y16g[:, q + j:q + j + 1], ident16)
        rrow = work.tile([1, 512], F16, tag="rrow", name="rrow", bufs=2)
        nc.vector.tensor_copy(rrow, prr)
        rbp = ps.tile([C, 512], F32, tag="GD"[kb % 2], name="rbp", bufs=1)
        nc.tensor.matmul(rbp, ones16[0:1, :], rrow, start=True, stop=True)
        ln = work.tile([C, 512], F16, tag="ln", bufs=2)
        with nc.allow_low_precision(reason="fp16 normalized activations"):
            nc.vector.tensor_mul(ln, mh, rbp)
        g1 = work.tile([C, 512], F16, tag="g1", bufs=2)
        nc.scalar.activation(g1, ln, gelu, bias=bet, scale=gam)
        g2 = work.tile([C, 512], F16, tag="g2", bufs=2)
        with nc.allow_low_precision(reason="fp16 pre-activation"):
            nc.vector.tensor_add(g2, g1, psc16)
        ob = work.tile([C, 512], F16, tag="ob")
        nc.scalar.activation(ob, g2, gelu, bias=scb, scale=1.0)
        nc.gpsimd.dma_start(d_out[:, 512 * kb:512 * (kb + 1)], ob)

    for g in range(len(GROUPS)):
        prev = GROUPS[g - 1] if g >= 1 else []
        for b in GROUPS[g]:
            blk_start(b)
        for i, b in enumerate(GROUPS[g]):
            blk_mid(b)
            if i < len(prev):
                blk_finish(prev[i])
        grp_chain(g)
    for b in GROUPS[-1]:
        blk_finish(b)

    ctx.close()


def _prep_inputs(inputs):
    """Host-side packing/precompute. Returns per-core in_maps."""
    f16 = np.float16
    f32 = np.float32
    x = np.asarray(inputs["x"], f32)
    br = np.asarray(inputs["basis_real"], f32)
    bi = np.asarray(inputs["basis_imag"], f32)
    awr = np.asarray(inputs["attn_w_r"], f32)
    awi = np.asarray(inputs["attn_w_i"], f32)
    abr = np.asarray(inputs["attn_b_r"], f32)
    abi = np.asarray(inputs["attn_b_i"], f32)
    alpha = np.asarray(inputs["alpha"], f32)
    mw = np.asarray(inputs["mixer_w"], f32)
    mb = np.asarray(inputs["mixer_b"], f32)
    gam = np.asarray(inputs["norm_gamma"], f32)
    bet = np.asarray(inputs["norm_beta"], f32)
    scw = np.asarray(inputs["shortcut_w"], f32)
    scb = np.asarray(inputs["shortcut_b"], f32)

    # qkv packed: [cin, 12, cout] with j = (i*2 + p)*2 + ab
    wqkv = np.empty((C, 12, C), f32)
    bqkv = np.empty((C, 4), f32)
    for i in range(3):
        wrT = awr[i].T  # [cin, cout]
        wiT = awi[i].T
        for p in range(2):
            h0 = slice(64 * p, 64 * p + 32)
            h1 = slice(64 * p + 32, 64 * p + 64)
            A = np.concatenate([wrT[:, h0], wiT[:, h0], wrT[:, h1], wiT[:, h1]], 1)
            Bm = np.concatenate([-wiT[:, h0], wrT[:, h0], -wiT[:, h1], wrT[:, h1]], 1)
            wqkv[:, (i * 2 + p) * 2, :] = A
            wqkv[:, (i * 2 + p) * 2 + 1, :] = Bm
            if i < 2:
                bqkv[:, i * 2 + p] = np.concatenate(
                    [abr[i][h0], abi[i][h0], abr[i][h1], abi[i][h1]])
    # v bias as rows (free axis in the direct-V^T layout)
    bvrow = np.empty((2, C), f32)
    for p in range(2):
        h0 = slice(64 * p, 64 * p + 32)
        h1 = slice(64 * p + 32, 64 * p + 64)
        bvrow[p] = np.concatenate([abr[2][h0], abi[2][h0], abr[2][h1], abi[2][h1]])

    # mixer: fold alpha and LN mean-centering
    W1 = mw * alpha[None, :]
    W2 = W1 - W1.mean(0, keepdims=True)
    b2 = mb - mb.mean()

    # o-proj folded with W2: yh[s, o] = sum_ch ot[h][ch, s] * wo2[ch, j, o]
    # wo2[:, part*4+h, :] = [worT[hs]; -+woiT[hs]] @ W2.T
    worT = awr[3].T
    woiT = awi[3].T
    wo2 = np.empty((64, 8, C), f32)
    for h in range(NH):
        hs = slice(32 * h, 32 * h + 32)
        wre = np.concatenate([worT[hs], -woiT[hs]], 0)   # [64, C]
        wim = np.concatenate([woiT[hs], worT[hs]], 0)
        wo2[:, 0 * 4 + h, :] = wre @ W2.T
        wo2[:, 1 * 4 + h, :] = wim @ W2.T
    byh = np.stack([W2 @ abr[3], W2 @ abi[3]], 0)  # [2, C]

    shared = {
        "wqkv": wqkv.astype(f16),
        "bqkv": bqkv,
        "bvrow": bvrow.astype(f16),
        "wo2": wo2.astype(f16),
        "byh": byh.astype(f16),
        "b2": b2,
        "gam": gam,
        "bet": bet,
        "scwT": np.ascontiguousarray(scw.T).astype(f16),
        "scb": scb,
    }
    in_maps = []
    for b in range(B):
        m = dict(shared)
        xb = x[b].reshape(C, HW)
        m["x16"] = np.ascontiguousarray(xb).astype(f16)
        m["xT16"] = np.ascontiguousarray(
            xb.T.reshape(NCH, 128, C).transpose(1, 0, 2)).astype(f16)
        br2 = br[b].reshape(HW, S)
        bi2 = bi[b].reshape(HW, S)
        br3 = br2.reshape(NCH, 128, S)
        bi3 = bi2.reshape(NCH, 128, S)
        bsb = np.concatenate([br3, bi3], 2).transpose(1, 0, 2)
        m["bs16"] = np.ascontiguousarray(bsb).astype(f16)
        # bsT[p, k, t, w] = catT[t*128+p, 512*k+w], catT = [br2.T; bi2.T]
        catT = np.empty((SCAT, HW), f16)
        catT[0:S] = br2.T
        catT[S:SCAT] = bi2.T
        bsT = catT.reshape(NT, 128, NBLK, 512).transpose(1, 2, 0, 3)
        m["bsT16"] = np.ascontiguousarray(bsT)
        in_maps.append(m)
    return in_maps


_CACHE = {}
PROFILE = False
LAST_RESULTS = None


def _get_module():
    if "nc" not in _CACHE:
        _CACHE["nc"] = build_module()
    return _CACHE["nc"]


def kernel(**inputs):
    nc = _get_module()
    in_maps = _prep_inputs(inputs)
    global LAST_RESULTS
    res = run_bass_kernel_spmd(nc, in_maps, core_ids=list(range(B)), trace=PROFILE)
    LAST_RESULTS = res
    out = np.stack([np.asarray(res.results[b]["out16"]).astype(np.float32)
                    .reshape(C, H, W) for b in range(B)])
    return out


# revision 56
# speedup vs baseline: 1.0204x; 1.0204x over previous
"""Trainium2 Bass kernel for nn_AttentionITBlock — v3 (zero on-chip transposes).

Contract: kernel(**inputs) takes FULL unsharded inputs, returns FULL output
(B, C, H, W) float32. Data-parallel over batch B=8, one batch per core.

Key ideas vs v2 (273us on HW):
  - Host ships the basis in BOTH contraction layouts as fp16: `bs` [hw, s]
    for the forward transform and `bsT` [s, hw] for the inverse. This
    removes all 648 on-chip PE transposes + 144 PSUM->SBUF copies that
    dominated v2's phase 3, at the cost of +21MB/core DMA (~51us at the
    measured 412GB/s/core, fully overlappable with attention + phase 3).
  - x also ships pre-transposed (xT) for the forward; the [C, HW] copy is
    streamed per-block for the shortcut. No PE transposes anywhere.
  - The inverse emits [hw, c] tiles (basis stationary, yh moving), so the
    LayerNorm reduction is along the FREE axis: one DVE square + one grouped
    tensor_reduce per 512-pixel block replaces v2's per-block ones-matmul /
    transpose / broadcast-matmul machinery (216 tiny PE ops gone). rstd is
    the same fp32 bit-hack + 2 Newton steps; normalization is a per-partition
    tensor_scalar; gamma/beta apply via pre-broadcast [128, 512] tiles.
  - Mixer W2 is folded into the attention output projection on the host
    (yh tiles come straight from ot), and biases (b2, scb, v-proj) fold into
    k=1 ones-row matmuls.
  - Attention: V^T produced directly by matmul (lhsT=Tcat tile), all 4 heads
    interleaved per token tile to fill the ACT exp latency, and the softmax
    1/sum uses reciprocal_approx_fast (1 DVE op) instead of the 1.9us
    iterative reciprocal.
  - Output is stored as [128, NBLK, 4, C] fp16 (partition-major pixels) and
    transposed back to [C, H, W] on the host.

PSUM banks: phase1 ph0-2 (3x [C,384]); attention po0-3 (4x [65,512]),
po64all [65,256], pl/pl64/pv rings; phase3 pm ping-pong (2x [128,4,128]w512),
psc ping-pong, pg staging for gamma/beta tiles.
"""

import sys

sys.path.insert(0, "/opt/trn_rl_repo")

import numpy as np

import concourse.bass as bass
import concourse.mybir as mybir
import concourse.tile as tile
from concourse import bacc
from concourse.bass_utils import run_bass_kernel_spmd

F32 = mybir.dt.float32
F16 = mybir.dt.float16
AF = mybir.ActivationFunctionType
I32 = mybir.dt.int32
ALU = mybir.AluOpType

B, C, H, W = 8, 128, 96, 96
HW = H * W                      # 9216
M1 = M2 = 24
S = M1 * M2                     # 576 tokens
SCAT = 2 * S                    # 1152 = re|im concatenated token axis
NH, DH = 4, 32
SCALE = 1.0 / np.sqrt(np.float32(DH))

NCH = HW // 128                 # 72 hw chunks of 128
NBLK = HW // 512                # 18 hw blocks of 512
NT = SCAT // 128                # 9 token tiles over the concatenated axis
TT = [(i * 128, min(128, S - i * 128)) for i in range((S + 127) // 128)]  # 5
MAGIC = 0x5F3759DF


DEBUG_DUMPS = False


def build_module(gelu=AF.Gelu_apprx_tanh):
    nc = bacc.Bacc("TRN2", target_bir_lowering=False, debug=False)

    d_x = nc.dram_tensor("x16", [C, HW], F16, kind="ExternalInput").ap()
    d_xT = nc.dram_tensor("xT16", [128, NCH, C], F16, kind="ExternalInput").ap()
    d_bs = nc.dram_tensor("bs16", [128, NCH, SCAT], F16, kind="ExternalInput").ap()
    d_bsT = nc.dram_tensor("bsT16", [128, NBLK, NT, 512], F16,
                           kind="ExternalInput").ap()
    d_wqkv = nc.dram_tensor("wqkv", [C, 12, C], F16, kind="ExternalInput").ap()
    d_bqkv = nc.dram_tensor("bqkv", [C, 4], F32, kind="ExternalInput").ap()
    d_bvrow = nc.dram_tensor("bvrow", [2, C], F16, kind="ExternalInput").ap()
    d_wo2 = nc.dram_tensor("wo2", [64, 8, C], F16, kind="ExternalInput").ap()
    d_byh = nc.dram_tensor("byh", [2, C], F16, kind="ExternalInput").ap()
    d_b2 = nc.dram_tensor("b2", [C], F32, kind="ExternalInput").ap()
    d_gam = nc.dram_tensor("gam", [C], F32, kind="ExternalInput").ap()
    d_bet = nc.dram_tensor("bet", [C], F32, kind="ExternalInput").ap()
    d_scw = nc.dram_tensor("scwT", [C, C], F16, kind="ExternalInput").ap()
    d_scb = nc.dram_tensor("scb", [C], F32, kind="ExternalInput").ap()
    d_out = nc.dram_tensor("out16", [C, HW], F16, kind="ExternalOutput").ap()
    dbg = None
    if DEBUG_DUMPS:
        dbg = {
            "Tcat": nc.dram_tensor("dbg_Tcat", [C, SCAT], F16,
                                   kind="ExternalOutput").ap(),
            "QP0": nc.dram_tensor("dbg_QP0", [C, S], F16,
                                  kind="ExternalOutput").ap(),
            "KP0": nc.dram_tensor("dbg_KP0", [C, S], F16,
                                  kind="ExternalOutput").ap(),
            "VH00": nc.dram_tensor("dbg_VH00", [128, 130], F16,
                                   kind="ExternalOutput").ap(),
            "OT": nc.dram_tensor("dbg_OT", [65, 4, S], F16,
                                 kind="ExternalOutput").ap(),
            "YH": nc.dram_tensor("dbg_YH", [128, NT, C], F16,
                                 kind="ExternalOutput").ap(),
        }

    with tile.TileContext(nc) as tc:
        _body(nc, tc, d_x, d_xT, d_bs, d_bsT, d_wqkv, d_bqkv, d_bvrow, d_wo2,
              d_byh, d_b2, d_gam, d_bet, d_scw, d_scb, d_out, gelu, dbg)
    nc.finalize()
    return nc


def _body(nc, tc, d_x, d_xT, d_bs, d_bsT, d_wqkv, d_bqkv, d_bvrow, d_wo2,
          d_byh, d_b2, d_gam, d_bet, d_scw, d_scb, d_out, gelu, dbg=None):
    from contextlib import ExitStack
    ctx = ExitStack()
    singles = ctx.enter_context(tc.tile_pool(name="singles", bufs=1))
    work = ctx.enter_context(tc.tile_pool(name="work", bufs=2))
    small = ctx.enter_context(tc.tile_pool(name="small", bufs=1))
    ps = ctx.enter_context(tc.tile_pool(name="ps", bufs=1, space="PSUM"))

    # ---------------- constants ----------------
    from concourse.masks import make_identity
    ident_f = work.tile([128, 128], F32, tag="identf", name="ident_f")
    make_identity(nc, ident_f)
    ident16 = singles.tile([128, 128], F16, tag="ident16")
    nc.vector.tensor_copy(ident16, ident_f)
    ones16 = singles.tile([128, 128], F16, tag="ones16")
    nc.vector.memset(ones16, 1.0)

    # ---------------- small weights (scalar queue; bulk loads go on sync) ----
    wqkv = singles.tile([C, 12, C], F16)
    nc.scalar.dma_start(wqkv, d_wqkv)
    wo2 = singles.tile([64, 8, C], F16)
    nc.scalar.dma_start(wo2, d_wo2)
    scw = singles.tile([C, C], F16, tag="scw")
    nc.scalar.dma_start(scw, d_scw)
    bqkv = singles.tile([C, 4], F32, tag="bqkv")
    nc.scalar.dma_start(bqkv, d_bqkv)
    # bias rows must each sit at partition 0 (matmul rhs with k=1)
    bvrow = []
    byh = []
    for p in range(2):
        t = singles.tile([1, C], F16, tag=f"bv{p}", name=f"bv{p}")
        nc.scalar.dma_start(t, d_bvrow[p:p + 1, :])
        bvrow.append(t)
        t = singles.tile([1, C], F16, tag=f"byh{p}", name=f"byh{p}")
        nc.scalar.dma_start(t, d_byh[p:p + 1, :])
        byh.append(t)
    def load_col(ap1d, nm):
        t = singles.tile([C, 1], F32, tag=nm, name=nm)
        nc.scalar.dma_start(t, ap1d[:, None])
        return t

    b2c = load_col(d_b2, "b2c")
    gam = load_col(d_gam, "gam")
    bet = load_col(d_bet, "bet")
    scb = load_col(d_scb, "scb")

    # ---------------- phase 1: forward transform ----------------
    # Stream xT + bs pieces of 4 chunks; 3 psum accumulators cover SCAT=1152.
    # The piece buffers are sized exactly like the phase-3 block buffers
    # (9216B / 1KB per partition), so both phases share one deep ring via
    # common tags — full temporal SBUF reuse.
    NP = 18
    per = NCH // NP  # 4
    RB = 12

    def load_piece(q):
        sl = slice(q * per, (q + 1) * per)
        xb = work.tile([128, per, C], F16, tag="xblk", bufs=RB, name="xpiece")
        nc.sync.dma_start(xb, d_xT[:, sl, :])
        bsb = work.tile([128, per, SCAT], F16, tag="bsT", bufs=RB,
                        name="bspiece")
        nc.sync.dma_start(bsb, d_bs[:, sl, :])
        return xb, bsb

    ph = [ps.tile([C, 384], F32, tag="ABC"[i], name=f"ph{i}", bufs=1)
          for i in range(3)]
    cur = load_piece(0)
    for q in range(NP):
        nxt = load_piece(q + 1) if q + 1 < NP else None
        xb, bsb = cur
        for j in range(per):
            k = per * q + j
            st = dict(start=(k == 0), stop=(k == NCH - 1))
            for i in range(3):
                nc.tensor.matmul(ph[i], xb[:, j, :],
                                 bsb[:, j, 384 * i:384 * (i + 1)], **st)
        cur = nxt

    Tcat = singles.tile([C, SCAT], F16, tag="Tcat")
    for i in range(3):
        nc.any.tensor_copy(Tcat[:, 384 * i:384 * (i + 1)], ph[i])
    if dbg:
        nc.gpsimd.dma_start(dbg["Tcat"], Tcat)

    # ---- prefetch phase-3 streams now (DMA queue drains during attention) --
    bsT_t = {}
    xblk_t = {}

    def prefetch_blk(k):
        t = work.tile([128, NT, 512], F16, tag="bsT", bufs=RB)
        nc.sync.dma_start(t, d_bsT[:, k, :, :])
        bsT_t[k] = t
        xb = work.tile([C, 512], F16, tag="xblk", bufs=RB)
        nc.sync.dma_start(xb, d_x[:, 512 * k:512 * (k + 1)])
        xblk_t[k] = xb

    for k in range(RB):
        prefetch_blk(k)

    # ---------------- phase 2: attention ----------------
    # PSUM: po0-3 in banks A-D, pl ring in E (2 bufs), bank G holds the po64
    # accumulators (cols 0:256; nothing else may write bank G while that
    # accumulation group is open — interleaved same-bank writes corrupt it)
    # plus the pre-loop pp64 staging (cols 256:512, temporally disjoint).
    # Bank H holds all loop-time 64-col transients: pl64 ping/pong (0:128),
    # pv ping/pong (128:384), rb64 ping/pong (384:512).
    gbank = ps.tile([128, 512], F32, tag="G", name="gbank", bufs=1)
    hbank = ps.tile([128, 512], F32, tag="H", name="hbank", bufs=1)

    # Projections: QP/KP per pair as [C, S] (packed [h0re|h0im|h1re|h1im]
    # rows); V^T directly via matmul into vh tiles [tsz, 130]
    # (= [h0(64) | one | h1(64) | one]).
    QP, KP = {}, {}
    for p in range(2):
        for i in range(2):  # 0=q, 1=k
            dst = singles.tile([C, S], F16, tag=f"qk{i}{p}", name=f"qk{i}{p}")
            iw = (i * 2 + p) * 2
            pp = ps.tile([C, 512], F32, tag="E", name="pp", bufs=2)
            nc.tensor.matmul(pp, wqkv[:, iw, :], Tcat[:, 0:512],
                             start=True, stop=False)
            nc.tensor.matmul(pp, wqkv[:, iw + 1, :], Tcat[:, S:S + 512],
                             start=False, stop=True)
            with nc.allow_low_precision(reason="fp16 qk"):
                nc.vector.tensor_scalar(dst[:, 0:512], pp,
                                        bqkv[:, i * 2 + p:i * 2 + p + 1],
                                        None, ALU.add)
            g0 = 256 + 64 * (i * 2 + p)  # 256..448, disjoint per proj
            pp64 = gbank[:, g0:g0 + 64]
            nc.tensor.matmul(pp64, wqkv[:, iw, :], Tcat[:, 512:576],
                             start=True, stop=False)
            nc.tensor.matmul(pp64, wqkv[:, iw + 1, :], Tcat[:, S + 512:S + 576],
                             start=False, stop=True)
            with nc.allow_low_precision(reason="fp16 qk"):
                nc.vector.tensor_scalar(dst[:, 512:576], pp64,
                                        bqkv[:, i * 2 + p:i * 2 + p + 1],
                                        None, ALU.add)
            (QP if i == 0 else KP)[p] = dst
    if dbg:
        nc.gpsimd.dma_start(dbg["QP0"], QP[0])
        nc.gpsimd.dma_start(dbg["KP0"], KP[0])

    vh = {}
    for p in range(2):
        for ti, (t0, tsz) in enumerate(TT):
            iw = (4 + p) * 2
            g0 = 128 + 128 * ((p * len(TT) + ti) % 2)
            pv = hbank[:, g0:g0 + 128]
            nc.tensor.matmul(pv[:tsz], Tcat[:, t0:t0 + tsz], wqkv[:, iw, :],
                             start=True, stop=False)
            nc.tensor.matmul(pv[:tsz], Tcat[:, S + t0:S + t0 + tsz],
                             wqkv[:, iw + 1, :], start=False, stop=False)
            nc.tensor.matmul(pv[:tsz], ones16[0:1, 0:tsz],
                             bvrow[p], start=False, stop=True)
            t = singles.tile([128, 130], F16, tag=f"vh{p}{ti}",
                             name=f"vh{p}{ti}")
            nc.vector.memset(t, 1.0)
            with nc.allow_low_precision(reason="fp16 v"):
                nc.vector.tensor_copy(t[:tsz, 0:64], pv[:tsz, 0:64])
                nc.scalar.copy(t[:tsz, 65:129], pv[:tsz, 64:128])
            vh[(p, ti)] = t
    if dbg:
        nc.gpsimd.dma_start(dbg["VH00"], vh[(0, 0)])

    # 4-head interleaved attention
    po = {h: ps.tile([65, 512], F32, tag="ABCD"[h], name=f"po{h}", bufs=1)
          for h in range(4)}
    for ti, (t0, tsz) in enumerate(TT):
        ats = {}
        for h in range(4):
            p, sub = h // 2, h % 2
            hsl = slice(64 * sub, 64 * sub + 64)
            tp = (64, 0) if sub else None
            at = work.tile([128, S], F16, tag="attnT", bufs=4)
            pl = ps.tile([128, 512], F32, tag="E", name="pl", bufs=2)
            nc.tensor.matmul(pl[:tsz], KP[p][hsl, t0:t0 + tsz],
                             QP[p][hsl, 0:512],
                             start=True, stop=True, tile_position=tp)
            nc.scalar.activation(at[:tsz, 0:512], pl[:tsz], AF.Exp,
                                 scale=float(SCALE))
            g0 = 64 * (h % 2)
            pl64 = hbank[:, g0:g0 + 64]
            nc.tensor.matmul(pl64[:tsz], KP[p][hsl, t0:t0 + tsz],
                             QP[p][hsl, 512:576],
                             start=True, stop=True, tile_position=tp)
            nc.scalar.activation(at[:tsz, 512:576], pl64[:tsz], AF.Exp,
                                 scale=float(SCALE))
            ats[h] = at
        for h in range(4):
            p, sub = h // 2, h % 2
            at = ats[h]
            lh = vh[(p, ti)][:tsz, 65 * sub:65 * sub + 65]
            nc.tensor.matmul(po[h], lh, at[:tsz, 0:512],
                             start=(ti == 0), stop=(ti == len(TT) - 1))
            # all 4 tails share bank G: exactly ONE start (whole bank goes
            # lazy-zero; each head's first write then stores) and ONE stop.
            nc.tensor.matmul(gbank[0:65, 64 * h:64 * h + 64], lh,
                             at[:tsz, 512:576],
                             start=(ti == 0 and h == 0),
                             stop=(ti == len(TT) - 1 and h == 3))

    # softmax 1/sum + normalize -> ot[h] [65, S] f16 (rows 0:64 normalized)
    ots = []
    for h in range(4):
        ot = singles.tile([65, S], F16, tag=f"ot{h}", name=f"ot{h}")
        with nc.allow_low_precision(reason="fp16 sums"):
            nc.vector.tensor_copy(ot[64:65, 0:512], po[h][64:65])
            nc.vector.tensor_copy(ot[64:65, 512:576],
                                  gbank[64:65, 64 * h:64 * h + 64])
        rb = ps.tile([64, 512], F32, tag="E", name="rb", bufs=2)
        nc.tensor.matmul(rb, ones16[64:65, 0:64], ot[64:65, 0:512],
                         start=True, stop=True, tile_position=(64, 0))
        g0 = 384 + 64 * (h % 2)
        rb64 = hbank[0:64, g0:g0 + 64]
        nc.tensor.matmul(rb64, ones16[64:65, 0:64], ot[64:65, 512:576],
                         start=True, stop=True, tile_position=(64, 0))
        rcp = work.tile([64, 512], F32, tag="rcp", name="rcp", bufs=2)
        nc.vector.reciprocal_approx_fast(rcp, rb)
        rcp64 = small.tile([64, 64], F32, tag=f"rcp64_{h}", bufs=1)
        nc.vector.reciprocal_approx_fast(rcp64, rb64)
        with nc.allow_low_precision(reason="fp16 softmax weights"):
            nc.vector.tensor_mul(ot[0:64, 0:512], po[h][0:64], rcp)
            nc.vector.tensor_mul(ot[0:64, 512:576],
                                 gbank[0:64, 64 * h:64 * h + 64], rcp64)
        ots.append(ot)
    if dbg:
        for h in range(4):
            nc.gpsimd.dma_start(dbg["OT"][:, h, :], ots[h])

    # yh fold: yh[s', o] = sum_h ot[h]^T @ wo2 (+ byh bias row), tiles over
    # the 1152 concat axis. Tile 4 straddles re|im at col 576: two 64-row
    # segments, the second writing psum partitions 64:128 (tile_position
    # auto-derives the M offset from out.base_partition()).
    yh = singles.tile([128, NT, C], F16, tag="yh")
    for t in range(NT):
        pyh = ps.tile([128, C], F32, tag="E", name="pyh", bufs=2)
        lo = 128 * t
        if lo < S < lo + 128:
            segs = [(0, lo, S - lo, 0), (S - lo, 0, lo + 128 - S, 1)]
        elif lo < S:
            segs = [(0, lo, 128, 0)]
        else:
            segs = [(0, lo - S, 128, 1)]
        for (m0, s0, mlen, part) in segs:
            out = pyh[m0:m0 + mlen, :]
            for h in range(NH):
                nc.tensor.matmul(out, ots[h][0:64, s0:s0 + mlen],
                                 wo2[:, part * 4 + h, :],
                                 start=(h == 0), stop=False)
            nc.tensor.matmul(out, ones16[0:1, 0:mlen], byh[part],
                             start=False, stop=True)
        with nc.allow_low_precision(reason="fp16 yh"):
            nc.vector.tensor_copy(yh[:, t, :], pyh)
    if dbg:
        nc.gpsimd.dma_start(dbg["YH"], yh)

    # ---------------- phase 3: inverse transform + LN tail ----------------
    # [c, hw] orientation: yh tiles are the (small) stationary weights, bsT
    # streams through the moving port at 2.4GHz. LN stats: one ones-column
    # matmul per block emits the variance directly as a ROW (at psum
    # partition 32q of bank H); the rstd bit-hack chain then runs on FOUR
    # blocks at once over a partition-strided [4, 512] view, and each
    # block's rstd row broadcasts to [C, 512] with a single k=1 matmul
    # (lhsT = ones row at the matching partition). gamma/beta/b2/scb ride
    # per-partition ACT bias/scale for free.
    GROUPS = [list(range(g, g + 3)) for g in range(0, NBLK, 3)]
    state = {}
    state2 = {}
    ydict = {}

    def blk_start(kb):
        if kb + RB < NBLK:
            prefetch_blk(kb + RB)
        bsT = bsT_t.pop(kb)
        xblk = xblk_t.pop(kb)
        pm = ps.tile([C, 512], F32, tag="AB"[kb % 2], name="pm", bufs=1)
        for t in range(NT):
            nc.tensor.matmul(pm, yh[:, t, :], bsT[:, t, :],
                             start=(t == 0), stop=(t == NT - 1))
        mh = work.tile([C, 512], F16, tag="mh", bufs=7)
        with nc.allow_low_precision(reason="fp16 mixed field"):
            nc.vector.tensor_scalar(mh, pm, b2c, None, ALU.add)
        sq = work.tile([C, 512], F16, tag="sq", bufs=5)
        with nc.allow_low_precision(reason="fp16 m^2 for variance"):
            nc.gpsimd.tensor_mul(sq, mh, mh)
        state[kb] = (mh, sq, xblk)

    def blk_mid(kb):
        mh, sq, xblk = state.pop(kb)
        psc = ps.tile([C, 512], F32, tag="C", name="psc", bufs=1)
        nc.tensor.matmul(psc, scw, xblk, start=True, stop=True)
        psc16 = work.tile([C, 512], F16, tag="psc16", bufs=7)
        nc.scalar.copy(psc16, psc)
        vcol = hbank[:, 4 * (kb % 3):4 * (kb % 3) + 4]
        for j in range(4):
            nc.tensor.matmul(vcol[:, j:j + 1], sq[:, 128 * j:128 * (j + 1)],
                             ones16[:, 0:1], start=(j == 0), stop=(j == 3))
        state2[kb] = (mh, psc16)

    def grp_chain(g):
        n = 4 * len(GROUPS[g])
        view = hbank[:, 0:n]
        v4 = work.tile([128, 12], F32, tag="cv4", bufs=1, name="cv4")[:, 0:n]
        w1 = work.tile([128, 12], F32, tag="cw1", bufs=1, name="cw1")[:, 0:n]
        w2 = work.tile([128, 12], F32, tag="cw2", bufs=1, name="cw2")[:, 0:n]
        w3 = work.tile([128, 12], F32, tag="cw3", bufs=1, name="cw3")[:, 0:n]
        nc.vector.tensor_scalar(v4, view, 1.0 / C, 1e-5, ALU.mult, ALU.add)
        nc.vector.tensor_scalar(w1.bitcast(I32), v4.bitcast(I32), 1, None,
                                ALU.logical_shift_right)
        nc.vector.tensor_scalar(w2.bitcast(I32), w1.bitcast(I32), -1, MAGIC,
                                ALU.mult, ALU.add)
        y = w2
        nc.vector.tensor_mul(w3, v4, y)
        nc.vector.tensor_mul(w1, w3, y)
        nc.vector.tensor_scalar(w3, w1, -0.5, 1.5, ALU.mult, ALU.add)
        yn = w1
        nc.vector.tensor_mul(yn, y, w3)
        y16g = work.tile([128, 12], F16, tag="y16g", bufs=2)
        with nc.allow_low_precision(reason="fp16 rstd"):
            nc.vector.tensor_copy(y16g[:, 0:n], yn)
        ydict[g] = y16g

    def blk_finish(kb):
        mh, psc16 = state2.pop(kb)
        q = 4 * (kb % 3)
        y16g = ydict[kb // 3]
        prr = ps.tile([1, 512], F16, tag="E", name="prr", bufs=2)
        for j in range(4):
            nc.tensor.transpose(prr[:, 128 * j:128 * (j + 1)],
                                y16g[:, q + j:q + j + 1], ident16)
        rrow = work.tile([1, 512], F16, tag="rrow", name="rrow", bufs=2)
        nc.vector.tensor_copy(rrow, prr)
        rbp = ps.tile([C, 512], F32, tag="GD"[kb % 2], name="rbp", bufs=1)
        nc.tensor.matmul(rbp, ones16[0:1, :], rrow, start=True, stop=True)
        ln = work.tile([C, 512], F16, tag="ln", bufs=2)
        with nc.allow_low_precision(reason="fp16 normalized activations"):
            nc.vector.tensor_mul(ln, mh, rbp)
        g1 = work.tile([C, 512], F16, tag="g1", bufs=2)
        nc.scalar.activation(g1, ln, gelu, bias=bet, scale=gam)
        g2 = work.tile([C, 512], F16, tag="g2", bufs=2)
        with nc.allow_low_precision(reason="fp16 pre-activation"):
            nc.vector.tensor_add(g2, g1, psc16)
        ob = work.tile([C, 512], F16, tag="ob")
        nc.scalar.activation(ob, g2, gelu, bias=scb, scale=1.0)
        nc.gpsimd.dma_start(d_out[:, 512 * kb:512 * (kb + 1)], ob)

    for g in range(len(GROUPS)):
        prev = GROUPS[g - 1] if g >= 1 else []
        for b in GROUPS[g]:
            blk_start(b)
        for i, b in enumerate(GROUPS[g]):
            blk_mid(b)
            if i < len(prev):
                blk_finish(prev[i])
        grp_chain(g)
    for b in GROUPS[-1]:
        blk_finish(b)

    ctx.close()


def _prep_inputs(inputs):
    """Host-side packing/precompute. Returns per-core in_maps."""
    f16 = np.float16
    f32 = np.float32
    x = np.asarray(inputs["x"], f32)
    br = np.asarray(inputs["basis_real"], f32)
    bi = np.asarray(inputs["basis_imag"], f32)
    awr = np.asarray(inputs["attn_w_r"], f32)
    awi = np.asarray(inputs["attn_w_i"], f32)
    abr = np.asarray(inputs["attn_b_r"], f32)
    abi = np.asarray(inputs["attn_b_i"], f32)
    alpha = np.asarray(inputs["alpha"], f32)
    mw = np.asarray(inputs["mixer_w"], f32)
    mb = np.asarray(inputs["mixer_b"], f32)
    gam = np.asarray(inputs["norm_gamma"], f32)
    bet = np.asarray(inputs["norm_beta"], f32)
    scw = np.asarray(inputs["shortcut_w"], f32)
    scb = np.asarray(inputs["shortcut_b"], f32)

    # qkv packed: [cin, 12, cout] with j = (i*2 + p)*2 + ab
    wqkv = np.empty((C, 12, C), f32)
    bqkv = np.empty((C, 4), f32)
    for i in range(3):
        wrT = awr[i].T  # [cin, cout]
        wiT = awi[i].T
        for p in range(2):
            h0 = slice(64 * p, 64 * p + 32)
            h1 = slice(64 * p + 32, 64 * p + 64)
            A = np.concatenate([wrT[:, h0], wiT[:, h0], wrT[:, h1], wiT[:, h1]], 1)
            Bm = np.concatenate([-wiT[:, h0], wrT[:, h0], -wiT[:, h1], wrT[:, h1]], 1)
            wqkv[:, (i * 2 + p) * 2, :] = A
            wqkv[:, (i * 2 + p) * 2 + 1, :] = Bm
            if i < 2:
                bqkv[:, i * 2 + p] = np.concatenate(
                    [abr[i][h0], abi[i][h0], abr[i][h1], abi[i][h1]])
    # v bias as rows (free axis in the direct-V^T layout)
    bvrow = np.empty((2, C), f32)
    for p in range(2):
        h0 = slice(64 * p, 64 * p + 32)
        h1 = slice(64 * p + 32, 64 * p + 64)
        bvrow[p] = np.concatenate([abr[2][h0], abi[2][h0], abr[2][h1], abi[2][h1]])

    # mixer: fold alpha and LN mean-centering
    W1 = mw * alpha[None, :]
    W2 = W1 - W1.mean(0, keepdims=True)
    b2 = mb - mb.mean()

    # o-proj folded with W2: yh[s, o] = sum_ch ot[h][ch, s] * wo2[ch, j, o]
    # wo2[:, part*4+h, :] = [worT[hs]; -+woiT[hs]] @ W2.T
    worT = awr[3].T
    woiT = awi[3].T
    wo2 = np.empty((64, 8, C), f32)
    for h in range(NH):
        hs = slice(32 * h, 32 * h + 32)
        wre = np.concatenate([worT[hs], -woiT[hs]], 0)   # [64, C]
        wim = np.concatenate([woiT[hs], worT[hs]], 0)
        wo2[:, 0 * 4 + h, :] = wre @ W2.T
        wo2[:, 1 * 4 + h, :] = wim @ W2.T
    byh = np.stack([W2 @ abr[3], W2 @ abi[3]], 0)  # [2, C]

    shared = {
        "wqkv": wqkv.astype(f16),
        "bqkv": bqkv,
        "bvrow": bvrow.astype(f16),
        "wo2": wo2.astype(f16),
        "byh": byh.astype(f16),
        "b2": b2,
        "gam": gam,
        "bet": bet,
        "scwT": np.ascontiguousarray(scw.T).astype(f16),
        "scb": scb,
    }
    in_maps = []
    for b in range(B):
        m = dict(shared)
        xb = x[b].reshape(C, HW)
        m["x16"] = np.ascontiguousarray(xb).astype(f16)
        m["xT16"] = np.ascontiguousarray(
            xb.T.reshape(NCH, 128, C).transpose(1, 0, 2)).astype(f16)
        br2 = br[b].reshape(HW, S)
        bi2 = bi[b].reshape(HW, S)
        br3 = br2.reshape(NCH, 128, S)
        bi3 = bi2.reshape(NCH, 128, S)
        bsb = np.concatenate([br3, bi3], 2).transpose(1, 0, 2)
        m["bs16"] = np.ascontiguousarray(bsb).astype(f16)
        # bsT[p, k, t, w] = catT[t*128+p, 512*k+w], catT = [br2.T; bi2.T]
        catT = np.empty((SCAT, HW), f16)
        catT[0:S] = br2.T
        catT[S:SCAT] = bi2.T
        bsT = catT.reshape(NT, 128, NBLK, 512).transpose(1, 2, 0, 3)
        m["bsT16"] = np.ascontiguousarray(bsT)
        in_maps.append(m)
    return in_maps


_CACHE = {}
PROFILE = False
LAST_RESULTS = None


def _get_module():
    if "nc" not in _CACHE:
        _CACHE["nc"] = build_module()
    return _CACHE["nc"]


def kernel(**inputs):
    nc = _get_module()
    in_maps = _prep_inputs(inputs)
    global LAST_RESULTS
    res = run_bass_kernel_spmd(nc, in_maps, core_ids=list(range(B)), trace=PROFILE)
    LAST_RESULTS = res
    out = np.stack([np.asarray(res.results[b]["out16"]).astype(np.float32)
                    .reshape(C, H, W) for b in range(B)])
    return out
